# revision 16
# baseline (speedup 1.0000x reference)
"""GumbelSlotSelector Trainium kernel.

Math (per row r of B*K rows, D=128, H=64):
  h = relu(x @ W1 + b1);  dlogit = h @ (W2[:,1]-W2[:,0]) + (b2[1]-b2[0])
  decision = 1.0 if dlogit + g1 - g0 > 0 else 0.0,  g_i = -log(-log(clip(u_i)))
  keep_probs = sigmoid(dlogit)
  fixup: rows (of K=64 slots) with no active slot activate their argmax(fix_u) slot.

Sharding: pure data-parallel over batch B=8192 -> 8 cores x 1024 batch rows
(65536 (b,k)-rows of 128 features per core).

Precision: slots/W1/w2d cast to fp16 on the host (halves the dominant HBM
traffic); slots pre-transposed to [D, R] so strip DMAs read contiguous
2KB/partition. fp32 PSUM accumulation. fix_u fp16 (fixup only breaks ties
in the never-taken all-inactive branch). gumbel_u stays fp32 (fp16 would
double decision flips). Outputs fp16, upcast on host.

Per-core dataflow (32 strips of 2048 rows; first/last strip DMA split into
512-row quarters so mm1 starts as soon as the first 128KB lands and the PE
tail after the last DMA byte is one quarter, not a full strip):
  DMA xT strip [128d, 2048] fp16 (sync HWDGE queue, 12-deep buffering)
  -> mm1 x4: stationary W1 [128,64]; quarter q -> psum partition half q%2,
     column chunk q//2 of a [128, 1024] psum tile (start=stop=True)
  -> relu(+b1) -> fp16 [128, 1024], split into two 512-col halves computed
     concurrently on DVE and ACT
  -> mm2 x2 (issued one strip late so the in-order PE queue never waits on
     relu): psum partition block a = strips 8a..8a+7 (PE tile position
     (0, 32a)); [128, 32] sliding-window stationary places w2d at window
     cols c,c+1 for half-strip pair 4k+2h; 16-matmul accumulation group.
  dl_ps partition c holds dlogits of rows 512c..512c+511. Final phase
  (gumbel threshold t1n = g0-g1, sigmoid keep, min-active fixup) runs per
  partition half; half A's ops are spread one per strip s=18.. so the DVE/
  ACT queues never stall the relu pipeline, and its output DMA rides the
  sync queue inside the x stream. Only half B trails the last mm2. ACT
  tables (Relu/Sigmoid/Ln) are pre-warmed at t=0. dec/keep live in one
  [128, 1024] fp16 tile -> one output DMA per half.
"""
import sys

sys.path.insert(0, "/opt/trn_rl_repo")
import numpy as np
from contextlib import ExitStack

import concourse.bacc as bacc
import concourse.tile as tile
from concourse import mybir, bass_utils
from concourse.bass import broadcast_tensor_aps
from concourse.bass_interp import get_hw_module

F32 = mybir.dt.float32
F16 = mybir.dt.float16
AF = mybir.ActivationFunctionType
ALU = mybir.AluOpType

B, K, D, H = 8192, 64, 128, 64
NCORES = 8
R = (B // NCORES) * K          # 65536 rows per core
SR = 2048                      # strip rows
NSTRIP = R // SR               # 32
CLIP_LO = 1e-10
CLIP_HI = float(np.float32(1.0 - 1e-7))

_CACHE = {}


def _build():
    nc = bacc.Bacc("TRN2", target_bir_lowering=False, debug=False,
                   num_devices=NCORES)
    x_d = nc.dram_tensor("xt16", [D, R], F16, kind="ExternalInput")
    gu_d = nc.dram_tensor("gu", [R, 2], F32, kind="ExternalInput")
    fu_d = nc.dram_tensor("fu16", [R], F16, kind="ExternalInput")
    w1_d = nc.dram_tensor("w1h", [D, H], F16, kind="ExternalInput")
    w2_d = nc.dram_tensor("w2p", [128, 126], F16, kind="ExternalInput")
    b1_d = nc.dram_tensor("b1c", [128, 1], F32, kind="ExternalInput")
    b2_d = nc.dram_tensor("b2dv", [128, 1], F32, kind="ExternalInput")
    out_d = nc.dram_tensor("out2", [128, 1024], F16, kind="ExternalOutput")

    with tile.TileContext(nc) as tc, ExitStack() as ctx:
        cpool = ctx.enter_context(tc.tile_pool(name="const", bufs=1))
        tpool = ctx.enter_context(tc.tile_pool(name="xt", bufs=20))
        rpool = ctx.enter_context(tc.tile_pool(name="relu", bufs=4))
        fpool = ctx.enter_context(tc.tile_pool(name="fin", bufs=1))
        ps_ht = ctx.enter_context(tc.tile_pool(name="psht", bufs=3, space="PSUM"))
        ps_dl = ctx.enter_context(tc.tile_pool(name="psdl", bufs=1, space="PSUM"))

        w1_sb = cpool.tile([D, H], F16)
        nc.scalar.dma_start(w1_sb[:], w1_d.ap())
        w2_sb = cpool.tile([128, 126], F16)
        nc.scalar.dma_start(w2_sb[:], w2_d.ap())
        b1_sb = cpool.tile([128, 1], F32)
        nc.scalar.dma_start(b1_sb[:], b1_d.ap())
        b2_sb = cpool.tile([128, 1], F32)
        nc.scalar.dma_start(b2_sb[:], b2_d.ap())

        # ACT table pre-warm: load Relu/Sigmoid/Ln tables during DMA fill so
        # none of them lands on the critical path later (Sigmoid otherwise
        # costs ~1.3us at the tail).
        tw = cpool.tile([128, 1], F32)
        nc.scalar.activation(tw[:], b2_sb[:], AF.Relu)
        nc.scalar.activation(tw[:], b2_sb[:], AF.Sigmoid)
        nc.scalar.activation(tw[:], b2_sb[:], AF.Ln)

        gu_sb = fpool.tile([128, 1024], F32)
        fu_sb = fpool.tile([128, 512], F16)
        dl_ps = ps_dl.tile([128, 512], F32)

        LAG = 2
        relus = []

        def emit_mm2(s):
            # dl group g = strips 16g..16g+15 -> psum partitions 64g..64g+63
            # (narrower PE tiles stream slower; 64-wide runs at full rate).
            # [128, 64] sliding-window stationary: w2d-upper at window col c,
            # w2d-lower at c+1 for half-strip pair c = 4(s%16)+2h. One
            # 32-matmul accumulation group per partition half, so each
            # half's final phase can start as soon as its group stops.
            g, m = divmod(s, 16)
            for h in range(2):
                c = 4 * m + 2 * h
                nc.tensor.matmul(
                    dl_ps[64 * g:64 * g + 64, :],
                    w2_sb[:, 62 - c:126 - c],
                    relus[s][:, 512 * h:512 * h + 512],
                    start=(m == 0 and h == 0), stop=(m == 15 and h == 1),
                    skip_group_check=True,
                )

        # out_sb cols 0:512 = decision, 512:1024 = keep_probs (fp16);
        # one DMA per partition half.
        out_sb = fpool.tile([128, 1024], F16)
        dec_sb = out_sb[:, 0:512]
        keep_sb = out_sb[:, 512:1024]
        rs = fpool.tile([128, 8], F16)
        fixm = fpool.tile([128, 512], F16)
        fmx = fpool.tile([128, 8], F16)
        t1n = fpool.tile([128, 512], F32)
        a0 = fpool.tile([128, 512], F32)
        a1 = fpool.tile([128, 512], F32)
        g0m = fpool.tile([128, 512], F32)
        g1m = fpool.tile([128, 512], F32)

        def phase_ops(half):
            """Final elementwise phase for one partition half, as a list of
            thunks so half A can be spread one op per strip."""
            p0, p1 = 64 * half, 64 * half + 64
            dec_v = dec_sb[p0:p1, :].rearrange("p (g k) -> p g k", k=64)
            fu_v = fu_sb[p0:p1, :].rearrange("p (g k) -> p g k", k=64)
            fixm_v = fixm[p0:p1, :].rearrange("p (g k) -> p g k", k=64)

            def t_sigmoid():
                nc.scalar.activation(keep_sb[p0:p1, :], dl_ps[p0:p1, :],
                                     AF.Sigmoid, bias=b2_sb[p0:p1, 0:1])

            def t_dec():
                nc.vector.scalar_tensor_tensor(
                    dec_sb[p0:p1, :], dl_ps[p0:p1, :], b2_sb[p0:p1, 0:1],
                    t1n[p0:p1, :], op0=ALU.add, op1=ALU.is_gt)

            def t_rs():
                with nc.allow_low_precision(reason="counts <= 64 exact fp16"):
                    nc.vector.reduce_sum(rs[p0:p1, :], dec_v,
                                         axis=mybir.AxisListType.X)

            def t_mult():
                # fixm *= (rs == 0), fused: one stt with rs broadcast
                rs_b = broadcast_tensor_aps(
                    fu_v, rs[p0:p1, :].rearrange("p (g o) -> p g o", o=1))[1]
                nc.vector.scalar_tensor_tensor(
                    fixm_v, rs_b, 0.0, fixm_v, op0=ALU.is_equal, op1=ALU.mult)

            def t_max():
                nc.vector.tensor_tensor(dec_sb[p0:p1, :], dec_sb[p0:p1, :],
                                        fixm[p0:p1, :], op=ALU.max)

            return [t_sigmoid, t_dec, t_rs, t_mult, t_max]

        pending = []

        for s in range(NSTRIP):
            xt_sb = tpool.tile([128, SR], F16)
            nc.sync.dma_start(xt_sb[:], x_d.ap()[:, s * SR:(s + 1) * SR])

            # quarter q covers global half-strip c = 4s+q: psum partition
            # half q%2, column chunk q//2
            ht_ps = ps_ht.tile([128, 1024], F32)
            for q in range(4):
                nc.tensor.matmul(
                    ht_ps[64 * (q % 2):64 * (q % 2) + 64,
                          512 * (q // 2):512 * (q // 2) + 512],
                    w1_sb[:],
                    xt_sb[:, q * 512:(q + 1) * 512],
                    start=True, stop=True,
                )
            relu_sb = rpool.tile([128, 1024], F16)
            nc.scalar.activation(relu_sb[:, 0:512], ht_ps[:, 0:512],
                                 AF.Relu, bias=b1_sb[:, 0:1])
            nc.vector.tensor_scalar(
                relu_sb[:, 512:1024], ht_ps[:, 512:1024], b1_sb[:, 0:1], 0.0,
                op0=ALU.add, op1=ALU.max)
            relus.append(relu_sb)

            if s >= LAG:
                emit_mm2(s - LAG)

            if s == 6:
                # gumbel/fixup inputs land mid-stream, off the x DMA queue
                nc.scalar.dma_start(
                    gu_sb[:].rearrange("p (s u) -> p s u", u=2),
                    gu_d.ap().rearrange("(p s) u -> p s u", p=128),
                )
                nc.scalar.dma_start(
                    fu_sb[:], fu_d.ap().rearrange("(p s) -> p s", p=128))
            # gumbel chain spread one op per strip to avoid engine bursts
            gu_v = gu_sb[:].rearrange("p (s u) -> p s u", u=2)
            if s == 8:
                nc.vector.tensor_scalar(a0[:], gu_v[:, :, 0], CLIP_LO,
                                        CLIP_HI, op0=ALU.max, op1=ALU.min)
            elif s == 9:
                nc.vector.tensor_scalar(a1[:], gu_v[:, :, 1], CLIP_LO,
                                        CLIP_HI, op0=ALU.max, op1=ALU.min)
            elif s == 10:
                nc.scalar.activation(a0[:], a0[:], AF.Ln)
            elif s == 11:
                nc.scalar.activation(a1[:], a1[:], AF.Ln)
            elif s == 12:
                # g_i = -log(-log(u_i)); g0m = log(-log u0) = -g0
                nc.scalar.activation(g0m[:], a0[:], AF.Ln, scale=-1.0)
            elif s == 13:
                nc.scalar.activation(g1m[:], a1[:], AF.Ln, scale=-1.0)
            elif s == 14:
                # t1n = g0 - g1, so decision = (dl + b2d) > t1n
                nc.vector.tensor_sub(t1n[:], g1m[:], g0m[:])
            elif s == 15:
                fu_vv = fu_sb[:].rearrange("p (g k) -> p g k", k=64)
                with nc.allow_low_precision(reason="max exact in fp16"):
                    nc.vector.reduce_max(fmx[:], fu_vv,
                                         axis=mybir.AxisListType.X)
            elif s == 16:
                # fixup candidate mask (fu == rowgroup max) is independent of
                # the decisions: precompute mid-stream for both halves
                fu_vv = fu_sb[:].rearrange("p (g k) -> p g k", k=64)
                fixm_vv = fixm[:].rearrange("p (g k) -> p g k", k=64)
                fmx_bb = broadcast_tensor_aps(
                    fu_vv, fmx[:].rearrange("p (g o) -> p g o", o=1))[1]
                nc.vector.tensor_tensor(fixm_vv, fu_vv, fmx_bb, op=ALU.is_ge)
            elif s == 18:
                pending = phase_ops(0)
            if pending:
                pending.pop(0)()
        for s in range(NSTRIP - LAG, NSTRIP):
            emit_mm2(s)
        for t in pending:
            t()
        for t in phase_ops(1):
            t()
        # output DMAs at the tail on the scalar queue: an in-order DMA queue
        # stalls on its head, so these must sit behind nothing else. Half A's
        # transfers run while half B's chain computes.
        for p0, p1 in ((0, 64), (64, 128)):
            nc.scalar.dma_start(out_d.ap()[p0:p1, 512:1024],
                                keep_sb[p0:p1, :])
            nc.scalar.dma_start(out_d.ap()[p0:p1, 0:512], dec_sb[p0:p1, :])

    nc.compile()
    nc.m = get_hw_module(nc.m)
    return nc


def kernel(slots, gumbel_u, fix_u, W1, b1, W2, b2, _trace=False):
    gumbel_u = np.ascontiguousarray(gumbel_u, np.float32)
    fu16 = np.ascontiguousarray(fix_u, np.float16)
    # fp16 + transpose: [B*K, D] -> [D, B*K] so each core's strip DMA reads
    # contiguous 2KB per partition
    x16t = np.ascontiguousarray(
        np.asarray(slots, np.float16).reshape(B * K, D).T)
    w1h = np.ascontiguousarray(W1, np.float16)
    W2 = np.ascontiguousarray(W2, np.float32)
    w2d = (W2[:, 1] - W2[:, 0]).astype(np.float16)
    b2d = np.float32(b2[1] - b2[0])

    # sliding-window mm2 stationary: window [:, 62-c:126-c] puts w2d-upper at
    # relative col c and w2d-lower at c+1
    w2p = np.zeros((128, 126), np.float16)
    w2p[0:H, 62] = w2d
    w2p[H:D, 63] = w2d
    b1c = np.tile(np.ascontiguousarray(b1, np.float32).reshape(H, 1), (2, 1))
    b2dv = np.full((128, 1), b2d, np.float32)

    if "nc" not in _CACHE:
        _CACHE["nc"] = _build()
    nc = _CACHE["nc"]

    bpc = B // NCORES
    in_maps = []
    for c in range(NCORES):
        in_maps.append({
            "xt16": np.ascontiguousarray(x16t[:, c * R:(c + 1) * R]),
            "gu": gumbel_u[c * bpc:(c + 1) * bpc].reshape(R, 2),
            "fu16": fu16[c * bpc:(c + 1) * bpc].reshape(R),
            "w1h": w1h, "w2p": w2p, "b1c": b1c, "b2dv": b2dv,
        })
    res = bass_utils.run_bass_kernel_spmd(
        nc, in_maps, core_ids=list(range(NCORES)), trace=_trace)
    _CACHE["last_result"] = res

    dec = np.concatenate(
        [res.results[c]["out2"][:, 0:512].astype(np.float32).reshape(bpc, K)
         for c in range(NCORES)], axis=0)
    keep = np.concatenate(
        [res.results[c]["out2"][:, 512:1024].astype(np.float32).reshape(bpc, K)
         for c in range(NCORES)], axis=0)
    return dec, keep
